# revision 11
# baseline (speedup 1.0000x reference)
"""Self-contained TRN2 Bass kernel for the COR Critic network.

kernel(**inputs) takes the FULL (unsharded) numpy inputs keyed as in
setup_inputs() and returns the FULL [131072, 1] float32 output.

Sharding: pure data parallel over 8 NeuronCores - the batch dim of
state/action is split into 8 equal shards; the (tiny) weights are
replicated. No collectives are needed; per-core outputs are
concatenated on the host.

Implementation notes (per 512-row super-tile, per core):
  - the whole network runs fused on-chip; no intermediate HBM traffic
  - matmul operands in fp16 (PSUM accumulation is fp32); LayerNorm
    statistics and normalization are computed in fp32
  - LayerNorm rstd via DVE Newton iterations (bit-trick seed), keeping
    the ACT engine inside a single activation-table set (tanh/relu)
  - sigmoid gates are folded into the next layer's weight rows on the
    host during marshalling
  - three-stage software pipeline (A / Bmid / Btail); the K=32 ripple-1
    matmuls ride inside ripple-2's j-loop as row-group-overlapped pairs
    (tile_position (0,0)/(32,0) run concurrently) so the PE stays dense
  - psA has 4 PSUM banks so the r2 j-loop never waits on its own ACT
    drain; Bmid transposes batch 4 chunks into one bank with a single
    ACT drain; q3 tail (relu/dot) runs on the otherwise-idle GpSimd
"""

import os

os.environ.setdefault("BASS_NEVER_TRACE", "1")

import numpy as np

import concourse.bacc as bacc
import concourse.bass as bass
import concourse.tile as tile
from concourse import mybir
from concourse.masks import make_identity

F32 = mybir.dt.float32
F32R = mybir.dt.float32r
F16 = mybir.dt.float16
I32 = mybir.dt.int32

# matmul-operand dtype: fp16 halves weight-load time (and enables FWL)
# at ~2e-4 relative rounding; all LayerNorm math stays fp32.
USE_FP16 = True
MMDT = F16 if USE_FP16 else F32R
MMNP = "float16" if USE_FP16 else "float32"
RSQRT_MAGIC = 0x5F3759DF

N_CORES = 8
B_CORE = 16384  # batch rows per core
T = 512         # super-tile batch rows
N_TILES = B_CORE // T
EPS = 1e-5


def build_nc():
    nc = bacc.Bacc("TRN2", target_bir_lowering=False, debug=False,
                   num_devices=N_CORES)

    # DRAM I/O (shapes match host-side pre-marshalled arrays)
    sa = nc.dram_tensor("sa", [N_TILES // 2, 64, T], MMDT, kind="ExternalInput").ap()
    w1 = nc.dram_tensor("w1", [64, 1024], MMDT, kind="ExternalInput").ap()
    b1 = nc.dram_tensor("b1", [128, 8], F32, kind="ExternalInput").ap()
    w2 = nc.dram_tensor("w2", [128, 8, 1024], MMDT, kind="ExternalInput").ap()
    b2 = nc.dram_tensor("b2", [128, 8], F32, kind="ExternalInput").ap()
    wq1 = nc.dram_tensor("wq1", [128, 8, 256], MMDT, kind="ExternalInput").ap()
    bq1 = nc.dram_tensor("bq1", [128, 256], F32, kind="ExternalInput").ap()
    l1g = nc.dram_tensor("l1g", [128, 2], F32, kind="ExternalInput").ap()
    l1b = nc.dram_tensor("l1b", [128, 2], F32, kind="ExternalInput").ap()
    wq2 = nc.dram_tensor("wq2", [128, 2, 128], MMDT, kind="ExternalInput").ap()
    bq2 = nc.dram_tensor("bq2", [128, 128], F32, kind="ExternalInput").ap()
    l2g = nc.dram_tensor("l2g", [128, 128], F32, kind="ExternalInput").ap()
    l2b = nc.dram_tensor("l2b", [128, 128], F32, kind="ExternalInput").ap()
    wq3 = nc.dram_tensor("wq3", [128, 128], F32, kind="ExternalInput").ap()
    bq3 = nc.dram_tensor("bq3", [128, 1], F32, kind="ExternalInput").ap()
    y = nc.dram_tensor("y", [128, 128], F32, kind="ExternalOutput").ap()

    AF = mybir.ActivationFunctionType
    OP = mybir.AluOpType

    with tile.TileContext(nc) as tc:
        with (
            tc.tile_pool(name="consts", bufs=1) as consts,
            tc.tile_pool(name="acts", bufs=2) as acts,
            tc.tile_pool(name="work", bufs=3) as work,
            tc.tile_pool(name="psA", bufs=4, space="PSUM") as psA,
            tc.tile_pool(name="psB", bufs=2, space="PSUM") as psB,
            tc.tile_pool(name="psC", bufs=2, space="PSUM") as psC,
        ):
            # ---------------- preamble: weights to SBUF ----------------
            # DMAs spread across engine queues so descriptor issue
            # (~0.6us each) parallelizes and the PE can start early.
            # scalar (ACT) queue carries NO DMA issues: the first tanh
            # must not sit behind descriptor setup. Critical path (first
            # rider chunk) = sa rows 0:32 + w1 rows 0:32 on gpsimd.
            sa2_0 = work.tile([64, T], MMDT, tag="sa_fm")
            w1_sb = consts.tile([64, 1024], MMDT, tag="w1")
            b1_sb = consts.tile([128, 8], F32, tag="b1")
            nc.gpsimd.dma_start(out=sa2_0[0:32, :], in_=sa[0, 0:32, :])
            nc.sync.dma_start(out=b1_sb, in_=b1)
            nc.gpsimd.dma_start(out=w1_sb[0:32, :], in_=w1[0:32, :])
            nc.sync.dma_start(out=sa2_0[32:64, :], in_=sa[0, 32:64, :])
            nc.gpsimd.dma_start(out=w1_sb[32:64, :], in_=w1[32:64, :])
            w2_sb = consts.tile([128, 8, 1024], MMDT, tag="w2")
            nc.sync.dma_start(out=w2_sb, in_=w2)
            b2_sb = consts.tile([128, 8], F32, tag="b2")
            nc.gpsimd.dma_start(out=b2_sb, in_=b2)
            wq1_sb = consts.tile([128, 8, 256], MMDT, tag="wq1")
            nc.gpsimd.dma_start(out=wq1_sb, in_=wq1)
            bq1_sb = consts.tile([128, 256], F32, tag="bq1")
            nc.gpsimd.dma_start(out=bq1_sb, in_=bq1)
            wq2_sb = consts.tile([128, 2, 128], MMDT, tag="wq2")
            nc.gpsimd.dma_start(out=wq2_sb, in_=wq2)
            wq3_sb = consts.tile([128, 128], F32, tag="wq3")
            nc.gpsimd.dma_start(out=wq3_sb, in_=wq3)
            l1g_sb = consts.tile([128, 2], F32, tag="l1g")
            nc.gpsimd.dma_start(out=l1g_sb, in_=l1g)
            l1b_sb = consts.tile([128, 2], F32, tag="l1b")
            nc.gpsimd.dma_start(out=l1b_sb, in_=l1b)
            bq2_sb = consts.tile([128, 128], F32, tag="bq2")
            nc.gpsimd.dma_start(out=bq2_sb, in_=bq2)
            l2g_sb = consts.tile([128, 128], F32, tag="l2g")
            nc.sync.dma_start(out=l2g_sb, in_=l2g)
            l2b_sb = consts.tile([128, 128], F32, tag="l2b")
            nc.sync.dma_start(out=l2b_sb, in_=l2b)
            bq3_sb = consts.tile([128, 1], F32, tag="bq3")
            nc.gpsimd.dma_start(out=bq3_sb, in_=bq3)

            y_all = consts.tile([128, 128], F32, tag="y_all")
            ident = consts.tile([128, 128], F32)
            make_identity(nc, ident)
            ident16 = consts.tile([128, 128], MMDT)
            nc.vector.tensor_copy(ident16, ident)
            magic = consts.tile([128, 4], I32)
            nc.vector.memset(magic, RSQRT_MAGIC)

            # Newton rsqrt on DVE (avoids ACT Sqrt: bad ULP + a table-set
            # swap against Tanh every tile). vars_ap: [128, n] variances.
            def rsqrt_dve(vars_ap, n):
                v = work.tile([128, 4], F32, tag="rsq_v")
                nc.vector.tensor_scalar_add(v[:, :n], in0=vars_ap, scalar1=EPS)
                ti = work.tile([128, 4], I32, tag="rsq_t")
                nc.vector.tensor_scalar(
                    ti[:, :n], in0=v[:, :n].bitcast(I32), scalar1=1,
                    scalar2=None, op0=OP.logical_shift_right)
                yn = work.tile([128, 4], F32, tag="rsq_y")
                nc.vector.tensor_sub(yn[:, :n].bitcast(I32), in0=magic[:, :n],
                                     in1=ti[:, :n])
                # 1 Newton step: seed err ~3.4% -> ~1.7e-3 worst-case on
                # rstd; tolerance is 2e-2 and the short DVE chain matters
                for _ in range(1):
                    a = work.tile([128, 4], F32, tag="rsq_a")
                    nc.vector.tensor_mul(a[:, :n], in0=yn[:, :n], in1=yn[:, :n])
                    nc.vector.scalar_tensor_tensor(
                        a[:, :n], in0=a[:, :n], scalar=-0.5, in1=v[:, :n],
                        op0=OP.mult, op1=OP.mult)
                    nc.vector.scalar_tensor_tensor(
                        yn[:, :n], in0=a[:, :n], scalar=1.5, in1=yn[:, :n],
                        op0=OP.add, op1=OP.mult)
                return yn

            # ------------- stage A: matmul-heavy front half -------------
            # Pair-structured. r1 matmuls (K=32, single-shot PSUM whose
            # slot frees only at tanh pace) are interleaved one-per-r2-
            # j-group so their PSUM slot is always free when they issue:
            # tile b's r1 rides tile a's r2; the NEXT pair's tile-a r1
            # rides tile b's r2. The two riders sit on row groups 1/0 and
            # execute concurrently on the PE.
            def r1_chunk(x1, sa2, m, j):
                ps = psA.tile([128, T], F32, tag="mm512")
                nc.tensor.matmul(
                    ps, w1_sb[32 * m:32 * (m + 1), j * 128:(j + 1) * 128],
                    sa2[32 * m:32 * (m + 1), :], start=True, stop=True,
                    tile_position=(32 * m, 0))
                nc.scalar.activation(x1[:, j, :], ps, AF.Tanh,
                                     bias=b1_sb[:, j:j + 1])

            def r2_q1(x1, riders):
                # ripple 2: x2 = tanh(W2f'.T @ x1 + b2)  [1024f, Tb]
                x2 = acts.tile([128, 8, T], MMDT, tag="x2")
                for j in range(8):
                    ps = psA.tile([128, T], F32, tag="mm512")
                    for k in range(8):
                        nc.tensor.matmul(
                            ps, w2_sb[:, k, j * 128:(j + 1) * 128],
                            x1[:, k, :], start=(k == 0), stop=(k == 7))
                    nc.scalar.activation(x2[:, j, :], ps, AF.Tanh,
                                         bias=b2_sb[:, j:j + 1])
                    for r in riders:
                        r1_chunk(*r, j)

                # q1 batch-major: z1 = x2.T @ Wq1' + bq1, then LN1 + norm
                z1sb = work.tile([128, 4, 256], F32, tag="z1sb", bufs=4)
                mv1 = work.tile([128, 4, 2], F32, tag="mv1", bufs=2)
                xn1 = work.tile([128, 4, 256], MMDT, tag="xn1", bufs=4)
                for cp in range(2):
                    zps2 = psB.tile([128, 2, 256], F32, tag="q1")
                    for ci in range(2):
                        c = 2 * cp + ci
                        for k in range(8):
                            nc.tensor.matmul(
                                zps2[:, ci, :], x2[:, k, c * 128:(c + 1) * 128],
                                wq1_sb[:, k, :], start=(k == 0), stop=(k == 7))
                        nc.vector.tensor_add(z1sb[:, c, :], in0=zps2[:, ci, :],
                                             in1=bq1_sb)
                        st = work.tile([128, 6], F32, tag="st1")
                        nc.vector.bn_stats(st, z1sb[:, c, :])
                        nc.vector.bn_aggr(mv1[:, c, :], st)
                    # per-pair rsqrt keeps the serial DVE chain short;
                    # the normalize itself runs on ACT: batch-major mu and
                    # rstd are per-partition, so Identity(rstd*z - mu*rstd)
                    rstd1 = rsqrt_dve(mv1[:, 2 * cp:2 * cp + 2, 1], 2)
                    nmr = work.tile([128, 2], F32, tag="nmr1")
                    nc.vector.scalar_tensor_tensor(
                        nmr, in0=mv1[:, 2 * cp:2 * cp + 2, 0], scalar=-1.0,
                        in1=rstd1[:, 0:2], op0=OP.mult, op1=OP.mult)
                    for ci in range(2):
                        c = 2 * cp + ci
                        nc.scalar.activation(
                            xn1[:, c, :], z1sb[:, c, :], AF.Identity,
                            bias=nmr[:, ci:ci + 1], scale=rstd1[:, ci:ci + 1])
                return xn1

            def stage_A_pair(p, x1_a, sa2):
                # resources for the NEXT pair (its tile-a r1 rides r2_b)
                nxt = None
                if p + 1 < N_TILES // 2:
                    sa2n = work.tile([64, T], MMDT, tag="sa_fm")
                    nc.sync.dma_start(out=sa2n, in_=sa[p + 1])
                    x1an = acts.tile([128, 8, T], MMDT, tag="x1", bufs=3)
                    nxt = (x1an, sa2n)

                x1_b = acts.tile([128, 8, T], MMDT, tag="x1", bufs=3)
                riders = [(x1_b, sa2, 1)]
                if nxt:
                    riders.append((nxt[0], nxt[1], 0))
                xn_a = r2_q1(x1_a, riders)
                xn_b = r2_q1(x1_b, [])
                return nxt, [xn_a, xn_b]

            # ------------- stage B mid: T1 + q2 + LN2 normalize -------------
            def stage_Bmid(t, xn1):
                # 4 transposed chunks land in one PSUM bank; one ACT op
                # per jf drains + relu + LN1 affine, so the transposes
                # never stall on per-chunk ACT pacing.
                h1T = work.tile([128, 2, T], MMDT, tag="h1T")
                for half in range(2):
                    for jf in range(2):
                        tp2 = psC.tile([128, 2, 128], MMDT, tag="tr4")
                        for ci in range(2):
                            c = 2 * half + ci
                            nc.tensor.transpose(
                                tp2[:, ci, :],
                                xn1[:, c, jf * 128:(jf + 1) * 128], ident16)
                        nc.scalar.activation(
                            h1T[:, jf, 256 * half:256 * (half + 1)], tp2,
                            AF.Relu, bias=l1b_sb[:, jf:jf + 1],
                            scale=l1g_sb[:, jf:jf + 1])

                # q2 batch-major directly: z2[b, o] (+bq2), LN2 stats
                z2T = work.tile([128, 4, 128], F32, tag="z2T", bufs=4)
                mv2 = work.tile([128, 4, 2], F32, tag="mv2", bufs=2)
                xn2 = work.tile([128, 4, 128], F32, tag="xn2", bufs=4)
                for cp in range(2):
                    zps2 = psB.tile([128, 2, 128], F32, tag="q1")
                    for ci in range(2):
                        c = 2 * cp + ci
                        for k in range(2):
                            nc.tensor.matmul(
                                zps2[:, ci, :], h1T[:, k, c * 128:(c + 1) * 128],
                                wq2_sb[:, k, :], start=(k == 0), stop=(k == 1))
                        nc.vector.tensor_add(z2T[:, c, :], in0=zps2[:, ci, :],
                                             in1=bq2_sb)
                        st2 = work.tile([128, 6], F32, tag="st2")
                        nc.vector.bn_stats(st2, z2T[:, c, :])
                        nc.vector.bn_aggr(mv2[:, c, :], st2)
                    rstd2 = rsqrt_dve(mv2[:, 2 * cp:2 * cp + 2, 1], 2)
                    for ci in range(2):
                        c = 2 * cp + ci
                        nc.vector.tensor_scalar(
                            xn2[:, c, :], in0=z2T[:, c, :],
                            scalar1=mv2[:, c, 0:1], scalar2=rstd2[:, ci:ci + 1],
                            op0=OP.subtract, op1=OP.mult)
                return xn2

            # ------------- stage B tail: q3 on DVE -------------
            # h2 = relu(xn2 * ln2_g + ln2_b); y = h2 . wq3 + bq3, with
            # bq3 folded in per-column so y_all columns are final the
            # moment their reduce lands (enables the split y flush).
            def stage_Btail(t, xn2):
                for c in range(4):
                    idx = t * 4 + c
                    h = work.tile([128, 128], F32, tag="hb")
                    nc.vector.tensor_mul(h, in0=xn2[:, c, :], in1=l2g_sb)
                    nc.vector.tensor_add(h, in0=h, in1=l2b_sb)
                    nc.vector.scalar_tensor_tensor(
                        h, in0=h, scalar=0.0, in1=wq3_sb,
                        op0=OP.max, op1=OP.mult)
                    nc.vector.reduce_sum(y_all[:, idx:idx + 1], h,
                                         axis=mybir.AxisListType.X)
                    nc.vector.tensor_scalar_add(
                        y_all[:, idx:idx + 1], in0=y_all[:, idx:idx + 1],
                        scalar1=bq3_sb)

            # flush y_all columns [lo, lo+64) to DRAM rows [lo, lo+64).
            # Transpose-mode outputs must land on PSUM partition 0; the
            # DMA AP handles the row placement in DRAM.
            y_sb = work.tile([64, 2, 128], F32, tag="ysb", bufs=1)

            def flush_y(lo):
                # carve the transpose target out of a psB-tagged bank
                zz = psB.tile([128, 2, 256], F32, tag="q1")
                yT = zz[:, 0, 0:128]
                h = lo // 64
                nc.tensor.transpose(yT[0:64, :], y_all[:, lo:lo + 64], ident)
                nc.scalar.copy(out=y_sb[:, h, :], in_=yT[0:64, :])
                nc.sync.dma_start(out=y[lo:lo + 64, :], in_=y_sb[:, h, :])

            # ---------------- software-pipelined batch loop ----------------
            NP = N_TILES // 2
            # prologue: pair 0's tile-a r1 runs standalone
            x1a_0 = acts.tile([128, 8, T], MMDT, tag="x1", bufs=3)
            for j in range(8):
                r1_chunk(x1a_0, sa2_0, 0, j)
            pend_a = (x1a_0, sa2_0)
            xn1q = {}
            xn2q = {}
            for p in range(NP):
                pend_a, xn1q[p] = stage_A_pair(p, *pend_a)
                if p >= 1:
                    stage_Btail(2 * (p - 1), xn2q[p - 1][0])
                    stage_Btail(2 * (p - 1) + 1, xn2q[p - 1][1])
                    del xn2q[p - 1]
                    if p == NP // 2:
                        # first 64 y columns are final; drain them early
                        # so the end-of-kernel tail only covers half
                        flush_y(0)
                xn2q[p] = (stage_Bmid(2 * p, xn1q[p][0]),
                           stage_Bmid(2 * p + 1, xn1q[p][1]))
                del xn1q[p]
            stage_Btail(2 * (NP - 1), xn2q[NP - 1][0])
            stage_Btail(2 * (NP - 1) + 1, xn2q[NP - 1][1])
            flush_y(64)

    nc.compile()
    return nc


def marshal_inputs(state, action, W1, b1, g1, W2, b2, g2,
                   Wq1, bq1, ln1_g, ln1_b, Wq2, bq2, ln2_g, ln2_b, Wq3, bq3):
    """Host-side layout marshalling (pure reshape/transpose/scale).

    The per-head sigmoid gates are folded into the next layer's weight
    rows here: (tanh(z)*sig(g)) @ W == tanh(z) @ (diag(sig(g)) W).

    Returns (shared weight map, per-core list of sa slabs)."""
    f32 = np.float32
    B = state.shape[0]
    assert B == N_CORES * B_CORE

    sa = np.concatenate([np.asarray(state, f32), np.asarray(action, f32)],
                        axis=1)  # [B, 32]
    # per-core: [N_TILES//2, 64, T] feature-major pair slabs
    sa_cores = []
    for cid in range(N_CORES):
        s = sa[cid * B_CORE:(cid + 1) * B_CORE]
        sa_cores.append(np.ascontiguousarray(
            s.reshape(N_TILES // 2, 2, T, 32).transpose(0, 1, 3, 2)
            .reshape(N_TILES // 2, 64, T)))

    sg1 = (1.0 / (1.0 + np.exp(-np.asarray(g1, np.float64)))).astype(f32)
    sg2 = (1.0 / (1.0 + np.exp(-np.asarray(g2, np.float64)))).astype(f32)

    # W1 [H=32, D=32, K=32] -> W1f [D=32, H*K=1024]
    w1f = np.asarray(W1, f32).transpose(1, 0, 2).reshape(32, 1024)
    w1f = np.ascontiguousarray(np.concatenate([w1f, w1f], axis=0))
    # W2 [H=32, D=1024, K=32] -> W2f [D=1024, H*K=1024] -> [128, 8, 1024]
    # rows pre-scaled by sig(g1) of the producing head
    w2f = (np.asarray(W2, f32).transpose(1, 0, 2).reshape(1024, 1024)
           * np.repeat(sg1, 32)[:, None])
    w2m = np.ascontiguousarray(
        w2f.reshape(8, 128, 1024).transpose(1, 0, 2))
    wq1f = np.asarray(Wq1, f32) * np.repeat(sg2, 32)[:, None]
    wq1m = np.ascontiguousarray(
        wq1f.reshape(8, 128, 256).transpose(1, 0, 2))
    wq2m = np.ascontiguousarray(
        np.asarray(Wq2, f32).reshape(2, 128, 128).transpose(1, 0, 2))
    wq3m = np.ascontiguousarray(np.tile(np.asarray(Wq3, f32).reshape(1, 128), (128, 1)))

    def pj(v, j):  # [j*128] vector -> [128, j]
        return np.ascontiguousarray(np.asarray(v, f32).reshape(j, 128).T)

    b1m = pj(np.asarray(b1, f32).reshape(1024), 8)
    b2m = pj(np.asarray(b2, f32).reshape(1024), 8)
    bq1m = np.ascontiguousarray(
        np.tile(np.asarray(bq1, f32)[None, :], (128, 1)))
    l1gm = pj(ln1_g, 2)
    l1bm = pj(ln1_b, 2)
    bq2m = np.ascontiguousarray(
        np.tile(np.asarray(bq2, f32)[None, :], (128, 1)))
    l2gm = np.ascontiguousarray(
        np.tile(np.asarray(ln2_g, f32)[None, :], (128, 1)))
    l2bm = np.ascontiguousarray(
        np.tile(np.asarray(ln2_b, f32)[None, :], (128, 1)))
    bq3m = np.full((128, 1), np.asarray(bq3, f32).reshape(()), f32)

    shared = dict(w1=w1f, b1=b1m, w2=w2m, b2=b2m,
                  wq1=wq1m, bq1=bq1m, l1g=l1gm, l1b=l1bm,
                  wq2=wq2m, bq2=bq2m, l2g=l2gm, l2b=l2bm,
                  wq3=wq3m, bq3=bq3m)
    if USE_FP16:
        for k in ("w1", "w2", "wq1", "wq2"):
            shared[k] = shared[k].astype(np.float16)
        sa_cores = [sc.astype(np.float16) for sc in sa_cores]
    return shared, sa_cores


def make_in_maps(**inputs):
    shared, sa_cores = marshal_inputs(**inputs)
    return [dict(shared, sa=sa_cores[c]) for c in range(N_CORES)]


def assemble_output(results):
    return np.concatenate(
        [results[c]["y"].reshape(B_CORE, 1) for c in range(N_CORES)], axis=0)


_NC_CACHE = []


def kernel(**inputs):
    from concourse.bass_utils import run_bass_kernel_spmd

    if not _NC_CACHE:
        _NC_CACHE.append(build_nc())
    nc = _NC_CACHE[0]
    in_maps = make_in_maps(**inputs)
    res = run_bass_kernel_spmd(nc, in_maps, core_ids=list(range(N_CORES)),
                               trace=False)
    return assemble_output(res.results)


# revision 12
# speedup vs baseline: 1.0480x; 1.0480x over previous
"""Self-contained TRN2 Bass kernel for the COR Critic network.

kernel(**inputs) takes the FULL (unsharded) numpy inputs keyed as in
setup_inputs() and returns the FULL [131072, 1] float32 output.

Sharding: pure data parallel over 8 NeuronCores - the batch dim of
state/action is split into 8 equal shards; the (tiny) weights are
replicated. No collectives are needed; per-core outputs are
concatenated on the host.

Implementation notes (per 512-row super-tile, per core):
  - the whole network runs fused on-chip; no intermediate HBM traffic
  - matmul operands in fp16 (PSUM accumulation is fp32); LayerNorm
    statistics and normalization are computed in fp32
  - LayerNorm rstd via DVE Newton iterations (bit-trick seed), keeping
    the ACT engine inside a single activation-table set (tanh/relu)
  - sigmoid gates are folded into the next layer's weight rows on the
    host during marshalling
  - three-stage software pipeline (A / Bmid / Btail); the K=32 ripple-1
    matmuls ride inside ripple-2's j-loop as row-group-overlapped pairs
    (tile_position (0,0)/(32,0) run concurrently) so the PE stays dense
  - psA has 4 PSUM banks so the r2 j-loop never waits on its own ACT
    drain; Bmid transposes batch 4 chunks into one bank with a single
    ACT drain; q3 tail (relu/dot) runs on the otherwise-idle GpSimd
"""

import os

os.environ.setdefault("BASS_NEVER_TRACE", "1")

import numpy as np

import concourse.bacc as bacc
import concourse.bass as bass
import concourse.tile as tile
from concourse import mybir
from concourse.masks import make_identity

F32 = mybir.dt.float32
F32R = mybir.dt.float32r
F16 = mybir.dt.float16
I32 = mybir.dt.int32

# matmul-operand dtype: fp16 halves weight-load time (and enables FWL)
# at ~2e-4 relative rounding; all LayerNorm math stays fp32.
USE_FP16 = True
MMDT = F16 if USE_FP16 else F32R
MMNP = "float16" if USE_FP16 else "float32"
RSQRT_MAGIC = 0x5F3759DF

N_CORES = 8
B_CORE = 16384  # batch rows per core
T = 512         # super-tile batch rows
N_TILES = B_CORE // T
EPS = 1e-5


def build_nc():
    nc = bacc.Bacc("TRN2", target_bir_lowering=False, debug=False,
                   num_devices=N_CORES)

    # DRAM I/O (shapes match host-side pre-marshalled arrays)
    sa = nc.dram_tensor("sa", [N_TILES // 2, 64, T], MMDT, kind="ExternalInput").ap()
    w1 = nc.dram_tensor("w1", [64, 1024], MMDT, kind="ExternalInput").ap()
    b1 = nc.dram_tensor("b1", [128, 8], F32, kind="ExternalInput").ap()
    w2 = nc.dram_tensor("w2", [128, 8, 1024], MMDT, kind="ExternalInput").ap()
    b2 = nc.dram_tensor("b2", [128, 8], F32, kind="ExternalInput").ap()
    wq1 = nc.dram_tensor("wq1", [128, 8, 256], MMDT, kind="ExternalInput").ap()
    bq1 = nc.dram_tensor("bq1", [128, 256], F32, kind="ExternalInput").ap()
    l1g = nc.dram_tensor("l1g", [128, 2], F32, kind="ExternalInput").ap()
    l1b = nc.dram_tensor("l1b", [128, 2], F32, kind="ExternalInput").ap()
    wq2 = nc.dram_tensor("wq2", [128, 2, 128], MMDT, kind="ExternalInput").ap()
    bq2 = nc.dram_tensor("bq2", [128, 128], F32, kind="ExternalInput").ap()
    l2g = nc.dram_tensor("l2g", [128, 128], F32, kind="ExternalInput").ap()
    l2b = nc.dram_tensor("l2b", [128, 128], F32, kind="ExternalInput").ap()
    wq3 = nc.dram_tensor("wq3", [128, 128], F32, kind="ExternalInput").ap()
    bq3 = nc.dram_tensor("bq3", [128, 1], F32, kind="ExternalInput").ap()
    y = nc.dram_tensor("y", [128, 128], F32, kind="ExternalOutput").ap()

    AF = mybir.ActivationFunctionType
    OP = mybir.AluOpType

    with tile.TileContext(nc) as tc:
        with (
            tc.tile_pool(name="consts", bufs=1) as consts,
            tc.tile_pool(name="acts", bufs=2) as acts,
            tc.tile_pool(name="work", bufs=3) as work,
            tc.tile_pool(name="psA", bufs=4, space="PSUM") as psA,
            tc.tile_pool(name="psB", bufs=2, space="PSUM") as psB,
            tc.tile_pool(name="psC", bufs=2, space="PSUM") as psC,
        ):
            # ---------------- preamble: weights to SBUF ----------------
            # DMAs spread across engine queues so descriptor issue
            # (~0.6us each) parallelizes and the PE can start early.
            # scalar (ACT) queue carries NO DMA issues: the first tanh
            # must not sit behind descriptor setup. Critical path (first
            # rider chunk) = sa rows 0:32 + w1 rows 0:32 on gpsimd.
            sa2_0 = work.tile([64, T], MMDT, tag="sa_fm")
            w1_sb = consts.tile([64, 1024], MMDT, tag="w1")
            b1_sb = consts.tile([128, 8], F32, tag="b1")
            nc.gpsimd.dma_start(out=sa2_0[0:32, :], in_=sa[0, 0:32, :])
            nc.sync.dma_start(out=b1_sb, in_=b1)
            nc.gpsimd.dma_start(out=w1_sb[0:32, :], in_=w1[0:32, :])
            nc.sync.dma_start(out=sa2_0[32:64, :], in_=sa[0, 32:64, :])
            nc.gpsimd.dma_start(out=w1_sb[32:64, :], in_=w1[32:64, :])
            w2_sb = consts.tile([128, 8, 1024], MMDT, tag="w2")
            nc.sync.dma_start(out=w2_sb, in_=w2)
            b2_sb = consts.tile([128, 8], F32, tag="b2")
            nc.gpsimd.dma_start(out=b2_sb, in_=b2)
            wq1_sb = consts.tile([128, 8, 256], MMDT, tag="wq1")
            nc.gpsimd.dma_start(out=wq1_sb, in_=wq1)
            bq1_sb = consts.tile([128, 256], F32, tag="bq1")
            nc.gpsimd.dma_start(out=bq1_sb, in_=bq1)
            wq2_sb = consts.tile([128, 2, 128], MMDT, tag="wq2")
            nc.gpsimd.dma_start(out=wq2_sb, in_=wq2)
            wq3_sb = consts.tile([128, 128], F32, tag="wq3")
            nc.gpsimd.dma_start(out=wq3_sb, in_=wq3)
            l1g_sb = consts.tile([128, 2], F32, tag="l1g")
            nc.gpsimd.dma_start(out=l1g_sb, in_=l1g)
            l1b_sb = consts.tile([128, 2], F32, tag="l1b")
            nc.gpsimd.dma_start(out=l1b_sb, in_=l1b)
            bq2_sb = consts.tile([128, 128], F32, tag="bq2")
            nc.gpsimd.dma_start(out=bq2_sb, in_=bq2)
            l2g_sb = consts.tile([128, 128], F32, tag="l2g")
            nc.sync.dma_start(out=l2g_sb, in_=l2g)
            l2b_sb = consts.tile([128, 128], F32, tag="l2b")
            nc.sync.dma_start(out=l2b_sb, in_=l2b)
            bq3_sb = consts.tile([128, 1], F32, tag="bq3")
            nc.gpsimd.dma_start(out=bq3_sb, in_=bq3)

            y_all = consts.tile([128, 128], F32, tag="y_all")
            ident = consts.tile([128, 128], F32)
            make_identity(nc, ident)
            ident16 = consts.tile([128, 128], MMDT)
            nc.vector.tensor_copy(ident16, ident)
            magic = consts.tile([128, 4], I32)
            nc.vector.memset(magic, RSQRT_MAGIC)

            # Newton rsqrt on DVE (avoids ACT Sqrt: bad ULP + a table-set
            # swap against Tanh every tile). vars_ap: [128, n] variances.
            def rsqrt_dve(vars_ap, n):
                v = work.tile([128, 4], F32, tag="rsq_v")
                nc.vector.tensor_scalar_add(v[:, :n], in0=vars_ap, scalar1=EPS)
                ti = work.tile([128, 4], I32, tag="rsq_t")
                nc.vector.tensor_scalar(
                    ti[:, :n], in0=v[:, :n].bitcast(I32), scalar1=1,
                    scalar2=None, op0=OP.logical_shift_right)
                yn = work.tile([128, 4], F32, tag="rsq_y")
                nc.vector.tensor_sub(yn[:, :n].bitcast(I32), in0=magic[:, :n],
                                     in1=ti[:, :n])
                # 1 Newton step: seed err ~3.4% -> ~1.7e-3 worst-case on
                # rstd; tolerance is 2e-2 and the short DVE chain matters
                for _ in range(1):
                    a = work.tile([128, 4], F32, tag="rsq_a")
                    nc.vector.tensor_mul(a[:, :n], in0=yn[:, :n], in1=yn[:, :n])
                    nc.vector.scalar_tensor_tensor(
                        a[:, :n], in0=a[:, :n], scalar=-0.5, in1=v[:, :n],
                        op0=OP.mult, op1=OP.mult)
                    nc.vector.scalar_tensor_tensor(
                        yn[:, :n], in0=a[:, :n], scalar=1.5, in1=yn[:, :n],
                        op0=OP.add, op1=OP.mult)
                return yn

            # ------------- stage A: matmul-heavy front half -------------
            # Pair-structured. r1 matmuls (K=32, single-shot PSUM whose
            # slot frees only at tanh pace) are interleaved one-per-r2-
            # j-group so their PSUM slot is always free when they issue:
            # tile b's r1 rides tile a's r2; the NEXT pair's tile-a r1
            # rides tile b's r2. The two riders sit on row groups 1/0 and
            # execute concurrently on the PE.
            def r1_chunk(x1, sa2, m, j):
                ps = psA.tile([128, T], F32, tag="mm512")
                nc.tensor.matmul(
                    ps, w1_sb[32 * m:32 * (m + 1), j * 128:(j + 1) * 128],
                    sa2[32 * m:32 * (m + 1), :], start=True, stop=True,
                    tile_position=(32 * m, 0))
                nc.scalar.activation(x1[:, j, :], ps, AF.Tanh,
                                     bias=b1_sb[:, j:j + 1])

            def r2_q1(x1, riders):
                # ripple 2: x2 = tanh(W2f'.T @ x1 + b2)  [1024f, Tb]
                x2 = acts.tile([128, 8, T], MMDT, tag="x2")
                for j in range(8):
                    ps = psA.tile([128, T], F32, tag="mm512")
                    for k in range(8):
                        nc.tensor.matmul(
                            ps, w2_sb[:, k, j * 128:(j + 1) * 128],
                            x1[:, k, :], start=(k == 0), stop=(k == 7))
                    nc.scalar.activation(x2[:, j, :], ps, AF.Tanh,
                                         bias=b2_sb[:, j:j + 1])
                    for r in riders:
                        r1_chunk(*r, j)

                # q1 batch-major: z1 = x2.T @ Wq1' + bq1, then LN1 + norm
                z1sb = work.tile([128, 4, 256], F32, tag="z1sb", bufs=4)
                mv1 = work.tile([128, 4, 2], F32, tag="mv1", bufs=2)
                xn1 = work.tile([128, 4, 256], MMDT, tag="xn1", bufs=4)
                for cp in range(2):
                    zps2 = psB.tile([128, 2, 256], F32, tag="q1")
                    for ci in range(2):
                        c = 2 * cp + ci
                        for k in range(8):
                            nc.tensor.matmul(
                                zps2[:, ci, :], x2[:, k, c * 128:(c + 1) * 128],
                                wq1_sb[:, k, :], start=(k == 0), stop=(k == 7))
                        nc.vector.tensor_add(z1sb[:, c, :], in0=zps2[:, ci, :],
                                             in1=bq1_sb)
                        st = work.tile([128, 6], F32, tag="st1")
                        nc.vector.bn_stats(st, z1sb[:, c, :])
                        nc.vector.bn_aggr(mv1[:, c, :], st)
                    # per-pair rsqrt+normalize keeps the serial DVE chain
                    # short so downstream transposes never wait on it
                    rstd1 = rsqrt_dve(mv1[:, 2 * cp:2 * cp + 2, 1], 2)
                    for ci in range(2):
                        c = 2 * cp + ci
                        nc.vector.tensor_scalar(
                            xn1[:, c, :], in0=z1sb[:, c, :],
                            scalar1=mv1[:, c, 0:1], scalar2=rstd1[:, ci:ci + 1],
                            op0=OP.subtract, op1=OP.mult)
                return xn1

            def stage_A_pair(p, x1_a, sa2):
                # resources for the NEXT pair (its tile-a r1 rides r2_b)
                nxt = None
                if p + 1 < N_TILES // 2:
                    sa2n = work.tile([64, T], MMDT, tag="sa_fm")
                    nc.sync.dma_start(out=sa2n, in_=sa[p + 1])
                    x1an = acts.tile([128, 8, T], MMDT, tag="x1", bufs=3)
                    nxt = (x1an, sa2n)

                x1_b = acts.tile([128, 8, T], MMDT, tag="x1", bufs=3)
                riders = [(x1_b, sa2, 1)]
                if nxt:
                    riders.append((nxt[0], nxt[1], 0))
                xn_a = r2_q1(x1_a, riders)
                xn_b = r2_q1(x1_b, [])
                return nxt, [xn_a, xn_b]

            # ------------- stage B mid: T1 + q2 + LN2 normalize -------------
            def stage_Bmid(t, xn1):
                # 4 transposed chunks land in one PSUM bank; one ACT op
                # per jf drains + relu + LN1 affine, so the transposes
                # never stall on per-chunk ACT pacing.
                h1T = work.tile([128, 2, T], MMDT, tag="h1T")
                for half in range(2):
                    for jf in range(2):
                        tp2 = psC.tile([128, 2, 128], MMDT, tag="tr4")
                        for ci in range(2):
                            c = 2 * half + ci
                            nc.tensor.transpose(
                                tp2[:, ci, :],
                                xn1[:, c, jf * 128:(jf + 1) * 128], ident16)
                        nc.scalar.activation(
                            h1T[:, jf, 256 * half:256 * (half + 1)], tp2,
                            AF.Relu, bias=l1b_sb[:, jf:jf + 1],
                            scale=l1g_sb[:, jf:jf + 1])

                # q2 batch-major directly: z2[b, o] (+bq2), LN2 stats
                z2T = work.tile([128, 4, 128], F32, tag="z2T", bufs=4)
                mv2 = work.tile([128, 4, 2], F32, tag="mv2", bufs=2)
                xn2 = work.tile([128, 4, 128], F32, tag="xn2", bufs=4)
                for cp in range(2):
                    zps2 = psB.tile([128, 2, 128], F32, tag="q1")
                    for ci in range(2):
                        c = 2 * cp + ci
                        for k in range(2):
                            nc.tensor.matmul(
                                zps2[:, ci, :], h1T[:, k, c * 128:(c + 1) * 128],
                                wq2_sb[:, k, :], start=(k == 0), stop=(k == 1))
                        nc.vector.tensor_add(z2T[:, c, :], in0=zps2[:, ci, :],
                                             in1=bq2_sb)
                        st2 = work.tile([128, 6], F32, tag="st2")
                        nc.vector.bn_stats(st2, z2T[:, c, :])
                        nc.vector.bn_aggr(mv2[:, c, :], st2)
                    rstd2 = rsqrt_dve(mv2[:, 2 * cp:2 * cp + 2, 1], 2)
                    for ci in range(2):
                        c = 2 * cp + ci
                        nc.vector.tensor_scalar(
                            xn2[:, c, :], in0=z2T[:, c, :],
                            scalar1=mv2[:, c, 0:1], scalar2=rstd2[:, ci:ci + 1],
                            op0=OP.subtract, op1=OP.mult)
                return xn2

            # ------------- stage B tail: q3 on DVE -------------
            # h2 = relu(xn2 * ln2_g + ln2_b); y = h2 . wq3 + bq3, with
            # bq3 folded in per-column so y_all columns are final the
            # moment their reduce lands (enables the split y flush).
            def stage_Btail(t, xn2):
                for c in range(4):
                    idx = t * 4 + c
                    h = work.tile([128, 128], F32, tag="hb")
                    nc.vector.tensor_mul(h, in0=xn2[:, c, :], in1=l2g_sb)
                    nc.vector.tensor_add(h, in0=h, in1=l2b_sb)
                    nc.vector.scalar_tensor_tensor(
                        h, in0=h, scalar=0.0, in1=wq3_sb,
                        op0=OP.max, op1=OP.mult)
                    nc.vector.reduce_sum(y_all[:, idx:idx + 1], h,
                                         axis=mybir.AxisListType.X)
                    nc.vector.tensor_scalar_add(
                        y_all[:, idx:idx + 1], in0=y_all[:, idx:idx + 1],
                        scalar1=bq3_sb)

            # flush y_all columns [lo, lo+64) to DRAM rows [lo, lo+64).
            # Transpose-mode outputs must land on PSUM partition 0; the
            # DMA AP handles the row placement in DRAM.
            y_sb = work.tile([64, 2, 128], F32, tag="ysb", bufs=1)

            def flush_y(lo):
                # carve the transpose target out of a psB-tagged bank
                zz = psB.tile([128, 2, 256], F32, tag="q1")
                yT = zz[:, 0, 0:128]
                h = lo // 64
                nc.tensor.transpose(yT[0:64, :], y_all[:, lo:lo + 64], ident)
                nc.scalar.copy(out=y_sb[:, h, :], in_=yT[0:64, :])
                nc.sync.dma_start(out=y[lo:lo + 64, :], in_=y_sb[:, h, :])

            # ---------------- software-pipelined batch loop ----------------
            NP = N_TILES // 2
            # prologue: pair 0's tile-a r1 runs standalone
            x1a_0 = acts.tile([128, 8, T], MMDT, tag="x1", bufs=3)
            for j in range(8):
                r1_chunk(x1a_0, sa2_0, 0, j)
            pend_a = (x1a_0, sa2_0)
            xn1q = {}
            xn2q = {}
            for p in range(NP):
                pend_a, xn1q[p] = stage_A_pair(p, *pend_a)
                if p >= 1:
                    stage_Btail(2 * (p - 1), xn2q[p - 1][0])
                    stage_Btail(2 * (p - 1) + 1, xn2q[p - 1][1])
                    del xn2q[p - 1]
                    if p == NP // 2:
                        # first 64 y columns are final; drain them early
                        # so the end-of-kernel tail only covers half
                        flush_y(0)
                xn2q[p] = (stage_Bmid(2 * p, xn1q[p][0]),
                           stage_Bmid(2 * p + 1, xn1q[p][1]))
                del xn1q[p]
            stage_Btail(2 * (NP - 1), xn2q[NP - 1][0])
            stage_Btail(2 * (NP - 1) + 1, xn2q[NP - 1][1])
            flush_y(64)

    nc.compile()
    return nc


def marshal_inputs(state, action, W1, b1, g1, W2, b2, g2,
                   Wq1, bq1, ln1_g, ln1_b, Wq2, bq2, ln2_g, ln2_b, Wq3, bq3):
    """Host-side layout marshalling (pure reshape/transpose/scale).

    The per-head sigmoid gates are folded into the next layer's weight
    rows here: (tanh(z)*sig(g)) @ W == tanh(z) @ (diag(sig(g)) W).

    Returns (shared weight map, per-core list of sa slabs)."""
    f32 = np.float32
    B = state.shape[0]
    assert B == N_CORES * B_CORE

    sa = np.concatenate([np.asarray(state, f32), np.asarray(action, f32)],
                        axis=1)  # [B, 32]
    # per-core: [N_TILES//2, 64, T] feature-major pair slabs
    sa_cores = []
    for cid in range(N_CORES):
        s = sa[cid * B_CORE:(cid + 1) * B_CORE]
        sa_cores.append(np.ascontiguousarray(
            s.reshape(N_TILES // 2, 2, T, 32).transpose(0, 1, 3, 2)
            .reshape(N_TILES // 2, 64, T)))

    sg1 = (1.0 / (1.0 + np.exp(-np.asarray(g1, np.float64)))).astype(f32)
    sg2 = (1.0 / (1.0 + np.exp(-np.asarray(g2, np.float64)))).astype(f32)

    # W1 [H=32, D=32, K=32] -> W1f [D=32, H*K=1024]
    w1f = np.asarray(W1, f32).transpose(1, 0, 2).reshape(32, 1024)
    w1f = np.ascontiguousarray(np.concatenate([w1f, w1f], axis=0))
    # W2 [H=32, D=1024, K=32] -> W2f [D=1024, H*K=1024] -> [128, 8, 1024]
    # rows pre-scaled by sig(g1) of the producing head
    w2f = (np.asarray(W2, f32).transpose(1, 0, 2).reshape(1024, 1024)
           * np.repeat(sg1, 32)[:, None])
    w2m = np.ascontiguousarray(
        w2f.reshape(8, 128, 1024).transpose(1, 0, 2))
    wq1f = np.asarray(Wq1, f32) * np.repeat(sg2, 32)[:, None]
    wq1m = np.ascontiguousarray(
        wq1f.reshape(8, 128, 256).transpose(1, 0, 2))
    wq2m = np.ascontiguousarray(
        np.asarray(Wq2, f32).reshape(2, 128, 128).transpose(1, 0, 2))
    wq3m = np.ascontiguousarray(np.tile(np.asarray(Wq3, f32).reshape(1, 128), (128, 1)))

    def pj(v, j):  # [j*128] vector -> [128, j]
        return np.ascontiguousarray(np.asarray(v, f32).reshape(j, 128).T)

    b1m = pj(np.asarray(b1, f32).reshape(1024), 8)
    b2m = pj(np.asarray(b2, f32).reshape(1024), 8)
    bq1m = np.ascontiguousarray(
        np.tile(np.asarray(bq1, f32)[None, :], (128, 1)))
    l1gm = pj(ln1_g, 2)
    l1bm = pj(ln1_b, 2)
    bq2m = np.ascontiguousarray(
        np.tile(np.asarray(bq2, f32)[None, :], (128, 1)))
    l2gm = np.ascontiguousarray(
        np.tile(np.asarray(ln2_g, f32)[None, :], (128, 1)))
    l2bm = np.ascontiguousarray(
        np.tile(np.asarray(ln2_b, f32)[None, :], (128, 1)))
    bq3m = np.full((128, 1), np.asarray(bq3, f32).reshape(()), f32)

    shared = dict(w1=w1f, b1=b1m, w2=w2m, b2=b2m,
                  wq1=wq1m, bq1=bq1m, l1g=l1gm, l1b=l1bm,
                  wq2=wq2m, bq2=bq2m, l2g=l2gm, l2b=l2bm,
                  wq3=wq3m, bq3=bq3m)
    if USE_FP16:
        for k in ("w1", "w2", "wq1", "wq2"):
            shared[k] = shared[k].astype(np.float16)
        sa_cores = [sc.astype(np.float16) for sc in sa_cores]
    return shared, sa_cores


def make_in_maps(**inputs):
    shared, sa_cores = marshal_inputs(**inputs)
    return [dict(shared, sa=sa_cores[c]) for c in range(N_CORES)]


def assemble_output(results):
    return np.concatenate(
        [results[c]["y"].reshape(B_CORE, 1) for c in range(N_CORES)], axis=0)


_NC_CACHE = []


def kernel(**inputs):
    from concourse.bass_utils import run_bass_kernel_spmd

    if not _NC_CACHE:
        _NC_CACHE.append(build_nc())
    nc = _NC_CACHE[0]
    in_maps = make_in_maps(**inputs)
    res = run_bass_kernel_spmd(nc, in_maps, core_ids=list(range(N_CORES)),
                               trace=False)
    return assemble_output(res.results)


# revision 13
# speedup vs baseline: 1.0556x; 1.0073x over previous
"""Self-contained TRN2 Bass kernel for the COR Critic network.

kernel(**inputs) takes the FULL (unsharded) numpy inputs keyed as in
setup_inputs() and returns the FULL [131072, 1] float32 output.

Sharding: pure data parallel over 8 NeuronCores - the batch dim of
state/action is split into 8 equal shards; the (tiny) weights are
replicated. No collectives are needed; per-core outputs are
concatenated on the host.

Implementation notes (per 512-row super-tile, per core):
  - the whole network runs fused on-chip; no intermediate HBM traffic
  - matmul operands in fp16 (PSUM accumulation is fp32); LayerNorm
    statistics and normalization are computed in fp32
  - LayerNorm rstd via DVE Newton iterations (bit-trick seed), keeping
    the ACT engine inside a single activation-table set (tanh/relu)
  - sigmoid gates are folded into the next layer's weight rows on the
    host during marshalling
  - three-stage software pipeline (A / Bmid / Btail); the K=32 ripple-1
    matmuls ride inside ripple-2's j-loop as row-group-overlapped pairs
    (tile_position (0,0)/(32,0) run concurrently) so the PE stays dense
  - psA has 4 PSUM banks so the r2 j-loop never waits on its own ACT
    drain; Bmid transposes batch 4 chunks into one bank with a single
    ACT drain; q3 tail (relu/dot) runs on the otherwise-idle GpSimd
"""

import os

os.environ.setdefault("BASS_NEVER_TRACE", "1")

import numpy as np

import concourse.bacc as bacc
import concourse.bass as bass
import concourse.tile as tile
from concourse import mybir
from concourse.masks import make_identity

F32 = mybir.dt.float32
F32R = mybir.dt.float32r
F16 = mybir.dt.float16
I32 = mybir.dt.int32

# matmul-operand dtype: fp16 halves weight-load time (and enables FWL)
# at ~2e-4 relative rounding; all LayerNorm math stays fp32.
USE_FP16 = True
MMDT = F16 if USE_FP16 else F32R
MMNP = "float16" if USE_FP16 else "float32"
RSQRT_MAGIC = 0x5F3759DF

N_CORES = 8
B_CORE = 16384  # batch rows per core
T = 512         # super-tile batch rows
N_TILES = B_CORE // T
EPS = 1e-5


def build_nc():
    nc = bacc.Bacc("TRN2", target_bir_lowering=False, debug=False,
                   num_devices=N_CORES)

    # DRAM I/O (shapes match host-side pre-marshalled arrays)
    sa = nc.dram_tensor("sa", [N_TILES // 2, 64, T], MMDT, kind="ExternalInput").ap()
    w1 = nc.dram_tensor("w1", [64, 1024], MMDT, kind="ExternalInput").ap()
    b1 = nc.dram_tensor("b1", [128, 8], F32, kind="ExternalInput").ap()
    w2 = nc.dram_tensor("w2", [128, 8, 1024], MMDT, kind="ExternalInput").ap()
    b2 = nc.dram_tensor("b2", [128, 8], F32, kind="ExternalInput").ap()
    wq1 = nc.dram_tensor("wq1", [128, 8, 256], MMDT, kind="ExternalInput").ap()
    bq1 = nc.dram_tensor("bq1", [128, 256], F32, kind="ExternalInput").ap()
    l1g = nc.dram_tensor("l1g", [128, 2], F32, kind="ExternalInput").ap()
    l1b = nc.dram_tensor("l1b", [128, 2], F32, kind="ExternalInput").ap()
    wq2 = nc.dram_tensor("wq2", [128, 2, 128], MMDT, kind="ExternalInput").ap()
    bq2 = nc.dram_tensor("bq2", [128, 128], F32, kind="ExternalInput").ap()
    l2g = nc.dram_tensor("l2g", [128, 128], F32, kind="ExternalInput").ap()
    l2b = nc.dram_tensor("l2b", [128, 128], F32, kind="ExternalInput").ap()
    wq3 = nc.dram_tensor("wq3", [128, 128], F32, kind="ExternalInput").ap()
    wq3c = nc.dram_tensor("wq3c", [128, 1], MMDT, kind="ExternalInput").ap()
    l2gc = nc.dram_tensor("l2gc", [128, 1], F32, kind="ExternalInput").ap()
    l2bc = nc.dram_tensor("l2bc", [128, 1], F32, kind="ExternalInput").ap()
    bq3 = nc.dram_tensor("bq3", [128, 1], F32, kind="ExternalInput").ap()
    y = nc.dram_tensor("y", [128, 128], F32, kind="ExternalOutput").ap()

    AF = mybir.ActivationFunctionType
    OP = mybir.AluOpType

    with tile.TileContext(nc) as tc:
        with (
            tc.tile_pool(name="consts", bufs=1) as consts,
            tc.tile_pool(name="acts", bufs=2) as acts,
            tc.tile_pool(name="work", bufs=3) as work,
            tc.tile_pool(name="psA", bufs=4, space="PSUM") as psA,
            tc.tile_pool(name="psB", bufs=2, space="PSUM") as psB,
            tc.tile_pool(name="psC", bufs=2, space="PSUM") as psC,
        ):
            # ---------------- preamble: weights to SBUF ----------------
            # DMAs spread across engine queues so descriptor issue
            # (~0.6us each) parallelizes and the PE can start early.
            # scalar (ACT) queue carries NO DMA issues: the first tanh
            # must not sit behind descriptor setup. Critical path (first
            # rider chunk) = sa rows 0:32 + w1 rows 0:32 on gpsimd.
            sa2_0 = work.tile([64, T], MMDT, tag="sa_fm")
            w1_sb = consts.tile([64, 1024], MMDT, tag="w1")
            b1_sb = consts.tile([128, 8], F32, tag="b1")
            nc.gpsimd.dma_start(out=sa2_0[0:32, :], in_=sa[0, 0:32, :])
            nc.sync.dma_start(out=b1_sb, in_=b1)
            nc.gpsimd.dma_start(out=w1_sb[0:32, :], in_=w1[0:32, :])
            nc.sync.dma_start(out=sa2_0[32:64, :], in_=sa[0, 32:64, :])
            nc.gpsimd.dma_start(out=w1_sb[32:64, :], in_=w1[32:64, :])
            w2_sb = consts.tile([128, 8, 1024], MMDT, tag="w2")
            nc.sync.dma_start(out=w2_sb, in_=w2)
            b2_sb = consts.tile([128, 8], F32, tag="b2")
            nc.gpsimd.dma_start(out=b2_sb, in_=b2)
            wq1_sb = consts.tile([128, 8, 256], MMDT, tag="wq1")
            nc.gpsimd.dma_start(out=wq1_sb, in_=wq1)
            bq1_sb = consts.tile([128, 256], F32, tag="bq1")
            nc.gpsimd.dma_start(out=bq1_sb, in_=bq1)
            wq2_sb = consts.tile([128, 2, 128], MMDT, tag="wq2")
            nc.gpsimd.dma_start(out=wq2_sb, in_=wq2)
            wq3_sb = consts.tile([128, 128], F32, tag="wq3")
            nc.gpsimd.dma_start(out=wq3_sb, in_=wq3)
            l1g_sb = consts.tile([128, 2], F32, tag="l1g")
            nc.gpsimd.dma_start(out=l1g_sb, in_=l1g)
            l1b_sb = consts.tile([128, 2], F32, tag="l1b")
            nc.gpsimd.dma_start(out=l1b_sb, in_=l1b)
            bq2_sb = consts.tile([128, 128], F32, tag="bq2")
            nc.gpsimd.dma_start(out=bq2_sb, in_=bq2)
            l2g_sb = consts.tile([128, 128], F32, tag="l2g")
            nc.sync.dma_start(out=l2g_sb, in_=l2g)
            l2b_sb = consts.tile([128, 128], F32, tag="l2b")
            nc.sync.dma_start(out=l2b_sb, in_=l2b)
            bq3_sb = consts.tile([128, 1], F32, tag="bq3")
            nc.gpsimd.dma_start(out=bq3_sb, in_=bq3)
            wq3c_sb = consts.tile([128, 1], MMDT, tag="wq3c")
            nc.gpsimd.dma_start(out=wq3c_sb, in_=wq3c)
            l2gc_sb = consts.tile([128, 1], F32, tag="l2gc")
            nc.gpsimd.dma_start(out=l2gc_sb, in_=l2gc)
            l2bc_sb = consts.tile([128, 1], F32, tag="l2bc")
            nc.gpsimd.dma_start(out=l2bc_sb, in_=l2bc)

            y_all = consts.tile([128, 128], F32, tag="y_all")
            ident = consts.tile([128, 128], F32)
            make_identity(nc, ident)
            ident16 = consts.tile([128, 128], MMDT)
            nc.vector.tensor_copy(ident16, ident)
            magic = consts.tile([128, 4], I32)
            nc.vector.memset(magic, RSQRT_MAGIC)

            # Newton rsqrt on DVE (avoids ACT Sqrt: bad ULP + a table-set
            # swap against Tanh every tile). vars_ap: [128, n] variances.
            def rsqrt_dve(vars_ap, n):
                v = work.tile([128, 4], F32, tag="rsq_v")
                nc.vector.tensor_scalar_add(v[:, :n], in0=vars_ap, scalar1=EPS)
                ti = work.tile([128, 4], I32, tag="rsq_t")
                nc.vector.tensor_scalar(
                    ti[:, :n], in0=v[:, :n].bitcast(I32), scalar1=1,
                    scalar2=None, op0=OP.logical_shift_right)
                yn = work.tile([128, 4], F32, tag="rsq_y")
                nc.vector.tensor_sub(yn[:, :n].bitcast(I32), in0=magic[:, :n],
                                     in1=ti[:, :n])
                # 1 Newton step: seed err ~3.4% -> ~1.7e-3 worst-case on
                # rstd; tolerance is 2e-2 and the short DVE chain matters
                for _ in range(1):
                    a = work.tile([128, 4], F32, tag="rsq_a")
                    nc.vector.tensor_mul(a[:, :n], in0=yn[:, :n], in1=yn[:, :n])
                    nc.vector.scalar_tensor_tensor(
                        a[:, :n], in0=a[:, :n], scalar=-0.5, in1=v[:, :n],
                        op0=OP.mult, op1=OP.mult)
                    nc.vector.scalar_tensor_tensor(
                        yn[:, :n], in0=a[:, :n], scalar=1.5, in1=yn[:, :n],
                        op0=OP.add, op1=OP.mult)
                return yn

            # ------------- stage A: matmul-heavy front half -------------
            # Pair-structured. r1 matmuls (K=32, single-shot PSUM whose
            # slot frees only at tanh pace) are interleaved one-per-r2-
            # j-group so their PSUM slot is always free when they issue:
            # tile b's r1 rides tile a's r2; the NEXT pair's tile-a r1
            # rides tile b's r2. The two riders sit on row groups 1/0 and
            # execute concurrently on the PE.
            def r1_chunk(x1, sa2, m, j):
                ps = psA.tile([128, T], F32, tag="mm512")
                nc.tensor.matmul(
                    ps, w1_sb[32 * m:32 * (m + 1), j * 128:(j + 1) * 128],
                    sa2[32 * m:32 * (m + 1), :], start=True, stop=True,
                    tile_position=(32 * m, 0))
                nc.scalar.activation(x1[:, j, :], ps, AF.Tanh,
                                     bias=b1_sb[:, j:j + 1])

            def r2_q1(x1, riders):
                # ripple 2: x2 = tanh(W2f'.T @ x1 + b2)  [1024f, Tb]
                x2 = acts.tile([128, 8, T], MMDT, tag="x2")
                for j in range(8):
                    ps = psA.tile([128, T], F32, tag="mm512")
                    for k in range(8):
                        nc.tensor.matmul(
                            ps, w2_sb[:, k, j * 128:(j + 1) * 128],
                            x1[:, k, :], start=(k == 0), stop=(k == 7))
                    nc.scalar.activation(x2[:, j, :], ps, AF.Tanh,
                                         bias=b2_sb[:, j:j + 1])
                    for r in riders:
                        r1_chunk(*r, j)

                # q1 batch-major: z1 = x2.T @ Wq1' + bq1, then LN1 + norm
                z1sb = work.tile([128, 4, 256], F32, tag="z1sb", bufs=4)
                mv1 = work.tile([128, 4, 2], F32, tag="mv1", bufs=2)
                xn1 = work.tile([128, 4, 256], MMDT, tag="xn1", bufs=4)
                for cp in range(2):
                    zps2 = psB.tile([128, 2, 256], F32, tag="q1")
                    for ci in range(2):
                        c = 2 * cp + ci
                        for k in range(8):
                            nc.tensor.matmul(
                                zps2[:, ci, :], x2[:, k, c * 128:(c + 1) * 128],
                                wq1_sb[:, k, :], start=(k == 0), stop=(k == 7))
                        nc.vector.tensor_add(z1sb[:, c, :], in0=zps2[:, ci, :],
                                             in1=bq1_sb)
                        st = work.tile([128, 6], F32, tag="st1")
                        nc.vector.bn_stats(st, z1sb[:, c, :])
                        nc.vector.bn_aggr(mv1[:, c, :], st)
                    # per-pair rsqrt+normalize keeps the serial DVE chain
                    # short so downstream transposes never wait on it
                    rstd1 = rsqrt_dve(mv1[:, 2 * cp:2 * cp + 2, 1], 2)
                    for ci in range(2):
                        c = 2 * cp + ci
                        nc.vector.tensor_scalar(
                            xn1[:, c, :], in0=z1sb[:, c, :],
                            scalar1=mv1[:, c, 0:1], scalar2=rstd1[:, ci:ci + 1],
                            op0=OP.subtract, op1=OP.mult)
                return xn1

            def stage_A_pair(p, x1_a, sa2):
                # resources for the NEXT pair (its tile-a r1 rides r2_b)
                nxt = None
                if p + 1 < N_TILES // 2:
                    sa2n = work.tile([64, T], MMDT, tag="sa_fm")
                    nc.sync.dma_start(out=sa2n, in_=sa[p + 1])
                    x1an = acts.tile([128, 8, T], MMDT, tag="x1", bufs=3)
                    nxt = (x1an, sa2n)

                x1_b = acts.tile([128, 8, T], MMDT, tag="x1", bufs=3)
                riders = [(x1_b, sa2, 1)]
                if nxt:
                    riders.append((nxt[0], nxt[1], 0))
                xn_a = r2_q1(x1_a, riders)
                xn_b = r2_q1(x1_b, [])
                return nxt, [xn_a, xn_b]

            # ------------- stage B mid: T1 + q2 + LN2 normalize -------------
            def stage_Bmid(t, xn1, out16=False):
                # 4 transposed chunks land in one PSUM bank; one ACT op
                # per jf drains + relu + LN1 affine, so the transposes
                # never stall on per-chunk ACT pacing.
                h1T = work.tile([128, 2, T], MMDT, tag="h1T")
                for half in range(2):
                    for jf in range(2):
                        tp2 = psC.tile([128, 2, 128], MMDT, tag="tr4")
                        for ci in range(2):
                            c = 2 * half + ci
                            nc.tensor.transpose(
                                tp2[:, ci, :],
                                xn1[:, c, jf * 128:(jf + 1) * 128], ident16)
                        nc.scalar.activation(
                            h1T[:, jf, 256 * half:256 * (half + 1)], tp2,
                            AF.Relu, bias=l1b_sb[:, jf:jf + 1],
                            scale=l1g_sb[:, jf:jf + 1])

                # q2 batch-major directly: z2[b, o] (+bq2), LN2 stats
                z2T = work.tile([128, 4, 128], F32, tag="z2T", bufs=4)
                mv2 = work.tile([128, 4, 2], F32, tag="mv2", bufs=2)
                if out16:
                    xn2 = work.tile([128, 4, 128], MMDT, tag="xn2h", bufs=2)
                else:
                    xn2 = work.tile([128, 4, 128], F32, tag="xn2", bufs=4)
                for cp in range(2):
                    zps2 = psB.tile([128, 2, 128], F32, tag="q1")
                    for ci in range(2):
                        c = 2 * cp + ci
                        for k in range(2):
                            nc.tensor.matmul(
                                zps2[:, ci, :], h1T[:, k, c * 128:(c + 1) * 128],
                                wq2_sb[:, k, :], start=(k == 0), stop=(k == 1))
                        nc.vector.tensor_add(z2T[:, c, :], in0=zps2[:, ci, :],
                                             in1=bq2_sb)
                        st2 = work.tile([128, 6], F32, tag="st2")
                        nc.vector.bn_stats(st2, z2T[:, c, :])
                        nc.vector.bn_aggr(mv2[:, c, :], st2)
                    rstd2 = rsqrt_dve(mv2[:, 2 * cp:2 * cp + 2, 1], 2)
                    for ci in range(2):
                        c = 2 * cp + ci
                        nc.vector.tensor_scalar(
                            xn2[:, c, :], in0=z2T[:, c, :],
                            scalar1=mv2[:, c, 0:1], scalar2=rstd2[:, ci:ci + 1],
                            op0=OP.subtract, op1=OP.mult)
                return xn2

            # ------------- stage B tail: q3 on DVE -------------
            # h2 = relu(xn2 * ln2_g + ln2_b); y = h2 . wq3 + bq3, with
            # bq3 folded in per-column so y_all columns are final the
            # moment their reduce lands (enables the split y flush).
            def stage_Btail(t, xn2):
                for c in range(4):
                    idx = t * 4 + c
                    h = work.tile([128, 128], F32, tag="hb")
                    nc.vector.tensor_mul(h, in0=xn2[:, c, :], in1=l2g_sb)
                    nc.vector.tensor_add(h, in0=h, in1=l2b_sb)
                    nc.vector.scalar_tensor_tensor(
                        h, in0=h, scalar=0.0, in1=wq3_sb,
                        op0=OP.max, op1=OP.mult)
                    nc.vector.reduce_sum(y_all[:, idx:idx + 1], h,
                                         axis=mybir.AxisListType.X)
                    nc.vector.tensor_scalar_add(
                        y_all[:, idx:idx + 1], in0=y_all[:, idx:idx + 1],
                        scalar1=bq3_sb)

            # PE-path tail for the last pair: transpose xn2 to feature-
            # major, relu-affine on ACT (l2g/l2b are per-partition there),
            # then a K=128 -> M=1 matmul against the wq3 column gives
            # y rows directly - no long serial DVE chain at kernel end.
            def stage_Btail_pe(t, xn2h):
                yq = psB.tile([128, 2, 256], F32, tag="q1")
                for half in range(2):
                    tp = psC.tile([128, 2, 128], MMDT, tag="tr4")
                    for ci in range(2):
                        nc.tensor.transpose(
                            tp[:, ci, :], xn2h[:, 2 * half + ci, :], ident16)
                    h2T = work.tile([128, 2, 128], MMDT, tag="h2T")
                    nc.scalar.activation(h2T, tp, AF.Relu,
                                         bias=l2bc_sb, scale=l2gc_sb)
                    for ci in range(2):
                        nc.tensor.matmul(
                            yq[0:1, half, ci * 128:(ci + 1) * 128],
                            wq3c_sb, h2T[:, ci, :], start=True, stop=True)
                ysb = work.tile([1, 512], F32, tag="ytail", bufs=2)
                nc.scalar.activation(ysb, yq[0:1, :, :], AF.Identity,
                                     bias=bq3_sb[0:1, 0:1])
                nc.sync.dma_start(out=y[4 * t:4 * t + 4, :], in_=ysb)

            # flush y_all columns [lo, hi) to DRAM rows [lo, hi).
            # Transpose-mode outputs must land on PSUM partition 0; the
            # DMA AP handles the row placement in DRAM.
            y_sb = work.tile([64, 2, 128], F32, tag="ysb", bufs=1)

            def flush_y(lo, hi):
                # carve the transpose target out of a psB-tagged bank
                zz = psB.tile([128, 2, 256], F32, tag="q1")
                yT = zz[:, 0, 0:128]
                h = lo // 64
                n = hi - lo
                nc.tensor.transpose(yT[0:n, :], y_all[:, lo:hi], ident)
                nc.scalar.copy(out=y_sb[0:n, h, :], in_=yT[0:n, :])
                nc.sync.dma_start(out=y[lo:hi, :], in_=y_sb[0:n, h, :])

            # ---------------- software-pipelined batch loop ----------------
            NP = N_TILES // 2
            # prologue: pair 0's tile-a r1 runs standalone
            x1a_0 = acts.tile([128, 8, T], MMDT, tag="x1", bufs=3)
            for j in range(8):
                r1_chunk(x1a_0, sa2_0, 0, j)
            pend_a = (x1a_0, sa2_0)
            xn1q = {}
            xn2q = {}
            for p in range(NP):
                pend_a, xn1q[p] = stage_A_pair(p, *pend_a)
                if p >= 1:
                    stage_Btail(2 * (p - 1), xn2q[p - 1][0])
                    stage_Btail(2 * (p - 1) + 1, xn2q[p - 1][1])
                    del xn2q[p - 1]
                    if p == NP // 2:
                        # first 64 y columns are final; drain them early
                        # so the end-of-kernel tail only covers half
                        flush_y(0, 64)
                last = p == NP - 1
                xn2q[p] = (stage_Bmid(2 * p, xn1q[p][0], out16=last),
                           stage_Bmid(2 * p + 1, xn1q[p][1], out16=last))
                del xn1q[p]
            stage_Btail_pe(2 * (NP - 1), xn2q[NP - 1][0])
            stage_Btail_pe(2 * (NP - 1) + 1, xn2q[NP - 1][1])
            flush_y(64, 120)

    nc.compile()
    return nc


def marshal_inputs(state, action, W1, b1, g1, W2, b2, g2,
                   Wq1, bq1, ln1_g, ln1_b, Wq2, bq2, ln2_g, ln2_b, Wq3, bq3):
    """Host-side layout marshalling (pure reshape/transpose/scale).

    The per-head sigmoid gates are folded into the next layer's weight
    rows here: (tanh(z)*sig(g)) @ W == tanh(z) @ (diag(sig(g)) W).

    Returns (shared weight map, per-core list of sa slabs)."""
    f32 = np.float32
    B = state.shape[0]
    assert B == N_CORES * B_CORE

    sa = np.concatenate([np.asarray(state, f32), np.asarray(action, f32)],
                        axis=1)  # [B, 32]
    # per-core: [N_TILES//2, 64, T] feature-major pair slabs
    sa_cores = []
    for cid in range(N_CORES):
        s = sa[cid * B_CORE:(cid + 1) * B_CORE]
        sa_cores.append(np.ascontiguousarray(
            s.reshape(N_TILES // 2, 2, T, 32).transpose(0, 1, 3, 2)
            .reshape(N_TILES // 2, 64, T)))

    sg1 = (1.0 / (1.0 + np.exp(-np.asarray(g1, np.float64)))).astype(f32)
    sg2 = (1.0 / (1.0 + np.exp(-np.asarray(g2, np.float64)))).astype(f32)

    # W1 [H=32, D=32, K=32] -> W1f [D=32, H*K=1024]
    w1f = np.asarray(W1, f32).transpose(1, 0, 2).reshape(32, 1024)
    w1f = np.ascontiguousarray(np.concatenate([w1f, w1f], axis=0))
    # W2 [H=32, D=1024, K=32] -> W2f [D=1024, H*K=1024] -> [128, 8, 1024]
    # rows pre-scaled by sig(g1) of the producing head
    w2f = (np.asarray(W2, f32).transpose(1, 0, 2).reshape(1024, 1024)
           * np.repeat(sg1, 32)[:, None])
    w2m = np.ascontiguousarray(
        w2f.reshape(8, 128, 1024).transpose(1, 0, 2))
    wq1f = np.asarray(Wq1, f32) * np.repeat(sg2, 32)[:, None]
    wq1m = np.ascontiguousarray(
        wq1f.reshape(8, 128, 256).transpose(1, 0, 2))
    wq2m = np.ascontiguousarray(
        np.asarray(Wq2, f32).reshape(2, 128, 128).transpose(1, 0, 2))
    wq3m = np.ascontiguousarray(np.tile(np.asarray(Wq3, f32).reshape(1, 128), (128, 1)))

    def pj(v, j):  # [j*128] vector -> [128, j]
        return np.ascontiguousarray(np.asarray(v, f32).reshape(j, 128).T)

    b1m = pj(np.asarray(b1, f32).reshape(1024), 8)
    b2m = pj(np.asarray(b2, f32).reshape(1024), 8)
    bq1m = np.ascontiguousarray(
        np.tile(np.asarray(bq1, f32)[None, :], (128, 1)))
    l1gm = pj(ln1_g, 2)
    l1bm = pj(ln1_b, 2)
    bq2m = np.ascontiguousarray(
        np.tile(np.asarray(bq2, f32)[None, :], (128, 1)))
    l2gm = np.ascontiguousarray(
        np.tile(np.asarray(ln2_g, f32)[None, :], (128, 1)))
    l2bm = np.ascontiguousarray(
        np.tile(np.asarray(ln2_b, f32)[None, :], (128, 1)))
    bq3m = np.full((128, 1), np.asarray(bq3, f32).reshape(()), f32)
    wq3cm = np.ascontiguousarray(np.asarray(Wq3, f32).reshape(128, 1))
    l2gcm = np.ascontiguousarray(np.asarray(ln2_g, f32).reshape(128, 1))
    l2bcm = np.ascontiguousarray(np.asarray(ln2_b, f32).reshape(128, 1))

    shared = dict(w1=w1f, b1=b1m, w2=w2m, b2=b2m,
                  wq1=wq1m, bq1=bq1m, l1g=l1gm, l1b=l1bm,
                  wq2=wq2m, bq2=bq2m, l2g=l2gm, l2b=l2bm,
                  wq3=wq3m, bq3=bq3m, wq3c=wq3cm, l2gc=l2gcm, l2bc=l2bcm)
    if USE_FP16:
        for k in ("w1", "w2", "wq1", "wq2", "wq3c"):
            shared[k] = shared[k].astype(np.float16)
        sa_cores = [sc.astype(np.float16) for sc in sa_cores]
    return shared, sa_cores


def make_in_maps(**inputs):
    shared, sa_cores = marshal_inputs(**inputs)
    return [dict(shared, sa=sa_cores[c]) for c in range(N_CORES)]


def assemble_output(results):
    return np.concatenate(
        [results[c]["y"].reshape(B_CORE, 1) for c in range(N_CORES)], axis=0)


_NC_CACHE = []


def kernel(**inputs):
    from concourse.bass_utils import run_bass_kernel_spmd

    if not _NC_CACHE:
        _NC_CACHE.append(build_nc())
    nc = _NC_CACHE[0]
    in_maps = make_in_maps(**inputs)
    res = run_bass_kernel_spmd(nc, in_maps, core_ids=list(range(N_CORES)),
                               trace=False)
    return assemble_output(res.results)
